# revision 27
# baseline (speedup 1.0000x reference)
"""Luong 'general' attention scores on Trainium2 (8 NeuronCores, Bass/Tile).

Reference math:
    proj[s,b,k]   = sum_h enc[s,b,h] * W[k,h] + bias[k]
    energies[b,s] = sum_k dec[b,k] * proj[s,b,k]
    out           = softmax(energies, axis=-1)          # [B, S]

Refactored: energies[b,s] = enc[s,b,:] . v[b,:] + (dec[b] . bias) with
v = dec @ W.  The bias dot-product is constant across s, so it cancels
exactly in softmax and is dropped.  This turns an O(S*B*H^2) matmul into
an O(S*B*H) stream over encoder_outputs (the only large tensor), which
is the true roofline of the problem.

Sharding: data-parallel over batch b.  Core i handles b in [4i, 4i+4):
  - enc slice [S, 4, H] (32 MiB) streamed once through SBUF,
  - v = dec_slice @ W on the PE (dec^T via PE transpose, W replicated),
  - v broadcast across partitions via PE one-hot matmul (no DRAM bounce),
  - energies: DVE/GPSIMD multiplies + ACT/DVE free-axis accumulates,
  - softmax stats in-place per b (GPSIMD partition_all_reduce for the
    cross-partition max/sum), single PE transpose, direct store.
"""

from contextlib import ExitStack

import numpy as np

S, B, H = 2048, 32, 1024
N_CORES = 8
BP = B // N_CORES  # batch rows per core
P = 128            # SBUF partitions
SB = S // P        # s-blocks of 128
KC = H // P        # k-chunks for dec @ W
SJ = 2             # s-blocks per enc DMA (1 MiB transfers)

_CACHE: dict = {}


def _build_nc(variant="full", stream_reps=1, sj=SJ, enc_bufs=6, gp_pairs=0,
              dve_accums=10):
    import concourse.bacc as bacc
    import concourse.bass as bass
    import concourse.bass_isa as bass_isa
    import concourse.mybir as mybir
    import concourse.tile as tile
    from concourse.masks import make_identity

    f32 = mybir.dt.float32
    nc = bacc.Bacc(None)

    enc = nc.declare_dram_parameter("enc", [S, BP, H], f32, isOutput=False)
    dec = nc.declare_dram_parameter("dec", [BP, H], f32, isOutput=False)
    w = nc.declare_dram_parameter("w", [H, H], f32, isOutput=False)
    probs = nc.declare_dram_parameter("probs", [BP, S], f32, isOutput=True)

    n_pairs_total = (SB // sj) * BP  # mults are done on sj-block groups
    gp_every = max(1, n_pairs_total // gp_pairs) if gp_pairs else 0
    n_blocks_total = BP * SB
    dve_every = max(1, n_blocks_total // dve_accums) if dve_accums else 0

    with tile.TileContext(nc) as tc, ExitStack() as ctx:
        singles = ctx.enter_context(tc.tile_pool(name="singles", bufs=1))
        wpool = ctx.enter_context(tc.tile_pool(name="wpool", bufs=1))
        encpool = ctx.enter_context(tc.tile_pool(name="encpool", bufs=enc_bufs))
        scratch = ctx.enter_context(tc.tile_pool(name="scratch", bufs=3))
        small = ctx.enter_context(tc.tile_pool(name="small", bufs=2))
        psum = ctx.enter_context(tc.tile_pool(name="psum", bufs=2, space="PSUM"))

        identity = singles.tile([P, P], f32)
        make_identity(nc, identity)

        v_bcast = singles.tile([P, BP, H], f32)
        if variant == "no_head":
            nc.vector.memset(v_bcast, 0.01)
        else:
            # ---- dec^T via PE transpose: decT[:, c, :] = dec[:, c*128:...]^T
            dec_sbuf = singles.tile([BP, H], f32)
            nc.sync.dma_start(out=dec_sbuf, in_=dec[:, :])
            decT = singles.tile([P, KC, BP], f32)
            for c in range(KC):
                pt = psum.tile([P, BP], f32, tag="psum_dec")
                nc.tensor.transpose(
                    pt, dec_sbuf[:, c * P : (c + 1) * P], identity[:BP, :BP]
                )
                nc.vector.tensor_copy(decT[:, c, :], pt)

            # ---- v = dec @ W on PE, accumulating over the 8 k-chunks ----
            wtiles = []
            for c in range(KC):
                wt = wpool.tile([P, H], f32, tag=f"w{c}")
                nc.sync.dma_start(out=wt, in_=w[c * P : (c + 1) * P, :])
                wtiles.append(wt)
            v_sbuf = singles.tile([BP, H], f32)
            for half in range(2):
                pv = psum.tile([BP, 512], f32, tag="psum_v")
                for c in range(KC):
                    nc.tensor.matmul(
                        pv[:, :],
                        decT[:, c, :],
                        wtiles[c][:, half * 512 : (half + 1) * 512],
                        start=(c == 0),
                        stop=(c == KC - 1),
                    )
                nc.vector.tensor_copy(
                    v_sbuf[:, half * 512 : (half + 1) * 512], pv[:, :]
                )

            # ---- broadcast v rows across partitions: move each row to
            # partition 0 (DMA), then a K=1 ones-matmul replicates it ----
            ones_row = singles.tile([1, P], f32)
            nc.vector.memset(ones_row, 1.0)
            vrow = singles.tile([1, BP, H], f32)
            for b in range(BP):
                nc.sync.dma_start(out=vrow[:, b, :], in_=v_sbuf[b : b + 1, :])
            for b in range(BP):
                for half in range(2):
                    pb = psum.tile([P, 512], f32, tag="psum_bc")
                    nc.tensor.matmul(
                        pb[:, :],
                        ones_row,
                        vrow[:, b, half * 512 : (half + 1) * 512],
                    )
                    nc.scalar.copy(
                        v_bcast[:, b, half * 512 : (half + 1) * 512], pb[:, :]
                    )

        # ---- main stream: energies_b[p, j] = enc[j*128+p, b, :] . v[b] ----
        # (one tile per b so per-b softmax stats don't false-serialize the
        # next b's accumulates through whole-tile dependency tracking)
        enc_v = enc[:, :, :].rearrange("(jo k p) b h -> jo p k b h", p=P, k=sj)
        energies_tiles = [
            singles.tile([P, SB], f32, name=f"energ{b}") for b in range(BP)
        ]
        pair_i = 0
        block_i = 0
        for _rep in range(stream_reps):
            for b in range(BP):
                vb = v_bcast[:, b, :]
                # repeat v along a step-0 middle dim to cover sj blocks
                vrep = bass.AP(
                    vb.tensor, vb.offset, [vb.ap[0], [0, sj], vb.ap[1]]
                )
                for jo in range(SB // sj):
                    et = encpool.tile([P, sj, H], f32, tag="enc")
                    nc.sync.dma_start(out=et, in_=enc_v[jo, :, :, b, :])
                    if variant == "dma_only":
                        continue
                    st = scratch.tile([P, sj, H], f32, tag="scr")
                    if variant != "accum_only":
                        on_gp = gp_pairs and (pair_i % gp_every == 0)
                        eng = nc.gpsimd if on_gp else nc.vector
                        eng.tensor_mul(st, et, vrep)
                    pair_i += 1
                    if variant == "mult_only":
                        continue
                    for k in range(sj):
                        col = jo * sj + k
                        on_dve = dve_accums and (block_i % dve_every == 0)
                        block_i += 1
                        if on_dve:
                            nc.vector.tensor_scalar(
                                out=st[:, k, :],
                                in0=st[:, k, :],
                                scalar1=1.0,
                                scalar2=None,
                                op0=mybir.AluOpType.mult,
                                op1=mybir.AluOpType.add,
                                accum_out=energies_tiles[b][:, col : col + 1],
                            )
                        else:
                            nc.scalar.activation(
                                out=st[:, k, :],
                                in_=st[:, k, :],
                                func=mybir.ActivationFunctionType.Copy,
                                accum_out=energies_tiles[b][:, col : col + 1],
                            )
            if variant in ("dma_only", "mult_only"):
                continue
            # ---- deferred softmax: stage-ordered across the 4 b's so the
            # slow GPSIMD all-reduces pipeline instead of stalling the
            # in-order DVE/ACT queues mid-stream ----
            ebs = [energies_tiles[b][:, :] for b in range(BP)]
            mxs = [small.tile([P, 1], f32, name=f"mx{b}") for b in range(BP)]
            sxs = [small.tile([P, 1], f32, name=f"sx{b}") for b in range(BP)]
            for b in range(BP):
                nc.vector.reduce_max(
                    out=mxs[b], in_=ebs[b], axis=mybir.AxisListType.X
                )
            for b in range(BP):
                nc.gpsimd.partition_all_reduce(
                    mxs[b], mxs[b], P, bass_isa.ReduceOp.max
                )
            for b in range(BP):
                nc.vector.tensor_scalar_mul(mxs[b], mxs[b], -1.0)
            for b in range(BP):
                nc.scalar.activation(
                    out=ebs[b],
                    in_=ebs[b],
                    func=mybir.ActivationFunctionType.Exp,
                    bias=mxs[b],
                    scale=1.0,
                    accum_out=sxs[b],
                )
            for b in range(BP):
                nc.gpsimd.partition_all_reduce(
                    sxs[b], sxs[b], P, bass_isa.ReduceOp.add
                )
            for b in range(BP):
                nc.vector.reciprocal(sxs[b], sxs[b])
            for b in range(BP):
                nc.vector.tensor_scalar_mul(ebs[b], ebs[b], sxs[b])
            for b in range(BP):
                pe = psum.tile([SB, P], f32, tag="psum_e")
                nc.tensor.transpose(pe, ebs[b], identity)
                eT = small.tile([SB, P], f32, tag="eT")
                nc.vector.tensor_copy(eT, pe)
                probs_b = bass.AP(probs, b * S, [[P, SB], [1, P]])
                nc.sync.dma_start(out=probs_b, in_=eT)

    nc.compile()
    return nc


def _get_nc():
    if "nc" not in _CACHE:
        _CACHE["nc"] = _build_nc()
    return _CACHE["nc"]


def _make_in_maps(rnn_outputs, encoder_outputs, W_attn):
    dec = np.ascontiguousarray(np.asarray(rnn_outputs, dtype=np.float32)[0])
    enc = np.asarray(encoder_outputs, dtype=np.float32)
    w = np.ascontiguousarray(np.asarray(W_attn, dtype=np.float32))
    in_maps = []
    for i in range(N_CORES):
        sl = slice(i * BP, (i + 1) * BP)
        in_maps.append(
            {
                "enc": np.ascontiguousarray(enc[:, sl, :]),
                "dec": np.ascontiguousarray(dec[sl, :]),
                "w": w,
            }
        )
    return in_maps


def run(rnn_outputs, encoder_outputs, W_attn, b_attn=None, trace=False, **trace_kwargs):
    """Run the kernel on 8 cores; returns (output [B, S], BassKernelResults)."""
    from concourse.bass_utils import run_bass_kernel_spmd

    nc = _get_nc()
    in_maps = _make_in_maps(rnn_outputs, encoder_outputs, W_attn)
    res = run_bass_kernel_spmd(
        nc, in_maps, list(range(N_CORES)), trace=trace, **trace_kwargs
    )
    out = np.concatenate([res.results[i]["probs"] for i in range(N_CORES)], axis=0)
    return out.astype(np.float32), res


def kernel(rnn_outputs, encoder_outputs, W_attn, b_attn=None):
    out, _ = run(rnn_outputs, encoder_outputs, W_attn, b_attn)
    return out
